# revision 28
# baseline (speedup 1.0000x reference)
"""Causal self-attention Trainium2 kernel (8 NeuronCores, tensor-parallel heads).

Problem: B=4, T=2048, C=1024, H=16, D=64 fp32.
  q,k,v = x@Wq+bq, x@Wk+bk, x@Wv+bv  (per-head causal softmax(qk^T/8) @ v) @ Wp + bp

Sharding: 2 heads per core (column-split Wq/Wk/Wv, row-split Wp). Each core
computes a partial output [B*T, C] in bf16; host sums the 8 partials in fp32
and adds bp.

All matmuls take bf16 inputs with fp32 PSUM accumulation; inputs are
quantized to bf16 on the host (~0.4% relative error against the 2e-2
tolerance). The kernel is software-pipelined around the PE engine:

  - Projections of batch b+1 and the out-projection of the previous
    attention block are queued as "filler" closures, popped one per
    attention tile, so the PE never idles while the ACT engine runs exp.
  - Within a tile, the next tile's S^T matmuls are emitted before this
    tile's P@V matmul (which must wait on exp), keeping the PE queue deep.
  - Engine balance: PE matmuls ~240us, ACT exp ~170us, DVE copies/norm
    ~170us, Pool (gpsimd) bias-adds/broadcasts ~70us.

Per-core dataflow:
  xT [C, B*T] bf16 streamed from DRAM (host pre-transposes x).
  Q^T/K^T/V^T [128, B*T] = w.T @ xT  (w slices [C,128] as stationary).
  V natural [tk,64] per head via PE transpose of V^T tiles; ones column
  appended -> V_aug [tk, 65] so P@V_aug also yields softmax row sums
  (v_aug is double-buffered across batches since projections of batch b+1
  overlap attention of batch b).
  S^T [tk,512] = K^T.T @ Q^T per (batch, tq-block, tk-tile); both heads run
  as K=64 matmuls on distinct PE row groups into one 2-bank PSUM tile.
  Causality: on block-diagonal tiles only the live columns [off*128, 512)
  are computed; the 128-wide triangle sub-block gets -50 added via one
  extra [128,128] matmul (L/E factorization), so softmax skips the
  max-subtraction pass (scores are O(1); exp stays in fp32 range).
  P^T = exp(S^T) via one double-wide ACT op per tile, PSUM -> SBUF bf16.
  Y_aug^T [65, 512] = V_aug.T @ P^T accumulated over tk tiles (live columns
  only on diagonal tiles); row 64 is the softmax denominator. Normalize:
  DVE copy of the sums row to SBUF -> reciprocal_approx_fast -> gpsimd
  partition_broadcast -> DVE multiply -> Y^T [128, RT] per block.
  out_part [512, 1024] = Y^T.T @ Wp_slice per block (as filler), copied
  PSUM -> SBUF bf16 on DVE and DMA'd out on the DVE queue.
"""

from collections import deque

import numpy as np

import concourse.tile as tile
from concourse import bacc, mybir
from concourse.bass_utils import run_bass_kernel_spmd

F32 = mybir.dt.float32
BF16 = mybir.dt.bfloat16

B, T, C, H = 4, 2048, 1024, 16
D = C // H  # 64
N_CORES = 8
RT = 512  # row-tile (tq block) size
KT = C // 128  # 8 contraction tiles for projections
NEG = -50.0  # causal mask additive constant (exp(-50+s) ~ 1e-20)

# pipeline feature flags (bisection knobs)
SBY = True  # emit next tile's S before this tile's Y
OPROJ_FILLER = True  # pop oproj pieces inside the tile loop (vs block start)
PROJ_FILLER = True  # queue next batch's projections as filler (vs inline)


def build_kernel(n_batches=B):
    nc = bacc.Bacc(None, target_bir_lowering=False, debug=False)
    rows = n_batches * T
    bt_rt = T // RT  # 4 tq blocks per batch

    xT_d = nc.dram_tensor("xT", [C, rows], BF16, kind="ExternalInput")
    wq_d = nc.dram_tensor("wq", [C, 128], BF16, kind="ExternalInput")
    wk_d = nc.dram_tensor("wk", [C, 128], BF16, kind="ExternalInput")
    wv_d = nc.dram_tensor("wv", [C, 128], BF16, kind="ExternalInput")
    wp_d = nc.dram_tensor("wp", [128, C], BF16, kind="ExternalInput")
    bq_d = nc.dram_tensor("bq", [128], F32, kind="ExternalInput")
    bk_d = nc.dram_tensor("bk", [128], F32, kind="ExternalInput")
    bv_d = nc.dram_tensor("bv", [128], F32, kind="ExternalInput")
    L_d = nc.dram_tensor("Lm", [128, 128], BF16, kind="ExternalInput")
    E_d = nc.dram_tensor("Em", [128, 4, 128], BF16, kind="ExternalInput")
    id_d = nc.dram_tensor("ident", [128, 64], BF16, kind="ExternalInput")
    t0_d = nc.dram_tensor("tri01", [128, 128], BF16, kind="ExternalInput")
    on_d = nc.dram_tensor(
        "onescol", [128, 2 * 2 * (T // 128)], BF16, kind="ExternalInput"
    )
    out_d = nc.dram_tensor("out", [rows, C], BF16, kind="ExternalOutput")

    with tile.TileContext(nc) as tc:
        with (
            nc.allow_low_precision(reason="bf16 intermediates are intentional"),
            tc.tile_pool(name="const", bufs=1) as const,
            tc.tile_pool(name="big", bufs=1) as big,
            # 8 bufs: xt slot reuse spans two batches, so a new load's WAR
            # deps are satisfied at emission time and the sync-queue head
            # never blocks (avoids cross-queue deadlock with out-DMAs)
            tc.tile_pool(name="xs", bufs=8) as xs,
            tc.tile_pool(name="vt", bufs=2) as vtp,
            tc.tile_pool(name="pt", bufs=4) as ptp,
            tc.tile_pool(name="yt", bufs=4) as ytp,
            tc.tile_pool(name="nrm", bufs=3) as nrm,
            tc.tile_pool(name="ob", bufs=3) as ob,
            # PSUM: 8 banks total.
            #   psS: S^T tiles [128,2,RT] f32 = 2 banks x 2 bufs = 4
            #   psY: Y accumulators [65,RT] f32 = 1 bank x 2 bufs = 2
            #   psF: filler work (proj accs, oproj tiles, V transposes) = 2
            tc.tile_pool(name="psS", bufs=2, space="PSUM") as psS,
            tc.tile_pool(name="psY", bufs=2, space="PSUM") as psY,
            tc.tile_pool(name="psF", bufs=2, space="PSUM") as psF,
        ):
            # ---- constants ----
            wq = const.tile([128, KT, 128], BF16)
            wk = const.tile([128, KT, 128], BF16)
            wv = const.tile([128, KT, 128], BF16)
            wp = const.tile([128, C], BF16)
            wq_src = wq_d.rearrange("(k p) m -> p k m", p=128)
            nc.sync.dma_start(wq[:], wq_src[:])
            biases = []
            for name, d in (("bq", bq_d), ("bk", bk_d), ("bv", bv_d)):
                t = const.tile([128, 1], F32, name=f"{name}_sb")
                nc.gpsimd.dma_start(t[:], d.rearrange("(p o) -> p o", o=1))
                biases.append(t)
            Lm = const.tile([128, 128], BF16)
            Em = const.tile([128, 4, 128], BF16)
            ident = const.tile([128, 64], BF16)
            tri01 = const.tile([128, 128], BF16)
            nc.gpsimd.dma_start(ident[:], id_d[:])
            nc.gpsimd.dma_start(tri01[:], t0_d[:])
            nc.gpsimd.dma_start(Lm[:], L_d[:])
            nc.gpsimd.dma_start(Em[:], E_d[:])
            nc.gpsimd.dma_start(wp[:], wp_d[:])

            # ---- whole-run big buffers ----
            n_rt_all = rows // RT
            qTs = [big.tile([128, RT], BF16, name=f"qT{i}") for i in range(n_rt_all)]
            kTs = [big.tile([128, RT], BF16, name=f"kT{i}") for i in range(n_rt_all)]
            n_vt = T // 128  # 16 v-tiles per batch per head
            # double-buffered: proj of batch b+1 overlaps attention of batch b
            v_augs = [
                big.tile([128, 2, n_vt, 65], BF16, name=f"vaug{i}") for i in range(2)
            ]
            on_src = on_d.rearrange("p (i h t o) -> p i h t o", i=2, h=2, o=1)
            for i in range(2):
                nc.gpsimd.dma_start(v_augs[i][:, :, :, 64:65], on_src[:, i])

            x_src = xT_d.rearrange("(k p) r -> p k r", p=128)
            xt0 = xs.tile([128, KT, RT], BF16, name="xt")
            nc.sync.dma_start(xt0[:, 0:2, :], x_src[:, 0:2, 0:RT])
            nc.sync.dma_start(xt0[:, 2:4, :], x_src[:, 2:4, 0:RT])
            nc.sync.dma_start(wk[:], wk_d.rearrange("(k p) m -> p k m", p=128))
            nc.sync.dma_start(xt0[:, 4:6, :], x_src[:, 4:6, 0:RT])
            nc.sync.dma_start(wv[:], wv_d.rearrange("(k p) m -> p k m", p=128))
            nc.sync.dma_start(xt0[:, 6:8, :], x_src[:, 6:8, 0:RT])

            # p-state warmup while the first DMAs land
            warm = psS.tile([128, 2, RT], F32, name="warm", tag="s")
            for _ in range(16):
                nc.tensor.matmul(warm[:, 0, :], Lm[:], Em[:], start=True, stop=True)

            # ---- filler machinery: closures giving the PE independent work
            # to chew on while ACT runs exp in the attention loop ----
            filler = deque()

            def pop_filler():
                if filler:
                    filler.popleft()()

            wqkv = (wq, wk, wv)

            def emit_proj_half1(xt, w):
                acc = psF.tile([128, RT], F32, name="acc", tag="f")
                for k in range(4):
                    nc.tensor.matmul(
                        acc[:], w[:, k, :], xt[:, k, :], start=(k == 0), stop=False
                    )
                return acc

            def emit_proj_half2(acc, xt, w, wi, bias, bsrc, rt):
                for k in range(4, KT):
                    nc.tensor.matmul(
                        acc[:], w[:, k, :], xt[:, k, :], start=False, stop=(k == KT - 1)
                    )
                if wi < 2:
                    dest = (qTs if wi == 0 else kTs)[bsrc * bt_rt + rt]
                    nc.vector.tensor_scalar_add(dest[:], acc[:], bias[:])
                else:
                    v_aug = v_augs[bsrc % 2]
                    vt_sb = vtp.tile([128, RT], BF16, name="vt_sb")
                    nc.vector.tensor_scalar_add(vt_sb[:], acc[:], bias[:])
                    for c in range(RT // 128):
                        vtile = rt * (RT // 128) + c
                        vps = psF.tile([128, 2, 64], BF16, name="vps", tag="f")
                        for h in range(2):
                            nc.tensor.transpose(
                                vps[:, h, :],
                                vt_sb[64 * h : 64 * h + 64, c * 128 : c * 128 + 128],
                                ident[64 * h : 64 * h + 64, :],
                            )
                            nc.vector.tensor_copy(
                                v_aug[:, h, vtile, 0:64], vps[:, h, :]
                            )

            def emit_proj_batch_inline(bsrc, rts=None):
                """Prologue only: batch 0's projections, emitted directly."""
                r0 = bsrc * T
                for rt in rts if rts is not None else range(bt_rt):
                    if bsrc == 0 and rt == 0:
                        xt = xt0
                    else:
                        xt = xs.tile([128, KT, RT], BF16, name="xt")
                        c0 = r0 + rt * RT
                        for kh in range(0, KT, 2):
                            nc.sync.dma_start(
                                xt[:, kh : kh + 2, :],
                                x_src[:, kh : kh + 2, c0 : c0 + RT],
                            )
                    for wi in range(3):
                        acc = emit_proj_half1(xt, wqkv[wi])
                        emit_proj_half2(acc, xt, wqkv[wi], wi, biases[wi], bsrc, rt)

            def queue_proj_batch(bsrc, rts=None):
                """Queue batch bsrc's projections (rts subset) as filler halves."""
                r0 = bsrc * T
                state = {}
                if rts is None:
                    rts = range(bt_rt)

                def load_xt(rt):
                    xt = xs.tile([128, KT, RT], BF16, name="xt")
                    c0 = r0 + rt * RT
                    for kh in range(0, KT, 2):
                        nc.sync.dma_start(
                            xt[:, kh : kh + 2, :],
                            x_src[:, kh : kh + 2, c0 : c0 + RT],
                        )
                    state[rt] = xt

                for rt in rts:
                    load_xt(rt)
                for rt in rts:
                    for wi in range(3):

                        def half1(rt=rt, wi=wi):
                            state[("acc", rt, wi)] = emit_proj_half1(
                                state[rt], wqkv[wi]
                            )

                        def half2(rt=rt, wi=wi):
                            emit_proj_half2(
                                state.pop(("acc", rt, wi)),
                                state[rt],
                                wqkv[wi],
                                wi,
                                biases[wi],
                                bsrc,
                                rt,
                            )

                        filler.append(half1)
                        filler.append(half2)

            def queue_oproj(yt, q0):
                for rr in range(RT // 128):

                    def piece(rr=rr, yt=yt, q0=q0):
                        osb = ob.tile([128, C], BF16, name="osb")
                        for nn in range(C // 512):
                            ops = psF.tile([128, 512], F32, name="ops", tag="f")
                            nc.tensor.matmul(
                                ops[:],
                                yt[:, rr * 128 : rr * 128 + 128],
                                wp[:, nn * 512 : nn * 512 + 512],
                                start=True,
                                stop=True,
                            )
                            if nn == 0:
                                nc.vector.tensor_copy(
                                    osb[:, nn * 512 : nn * 512 + 512], ops[:]
                                )
                            else:
                                nc.scalar.copy(
                                    osb[:, nn * 512 : nn * 512 + 512], ops[:]
                                )
                        nc.sync.dma_start(
                            out_d[q0 + rr * 128 : q0 + rr * 128 + 128, :], osb[:]
                        )

                    filler.append(piece)

            # ---- prologue: batch 0 projections ----
            with nc.named_scope("proj0"):
                emit_proj_batch_inline(0, rts=(0, 1))

            pending_oproj = []

            for b in range(n_batches):
                if PROJ_FILLER:
                    queue_proj_batch(b, rts=(2, 3))
                    if b + 1 < n_batches:
                        queue_proj_batch(b + 1, rts=(0, 1))
                elif b > 0:
                    with nc.named_scope(f"proj{b}"):
                        emit_proj_batch_inline(b)
                v_aug = v_augs[b % 2]
                r0 = b * T
                # last batch runs tqb order 1,2,3,0 so the kernel tail is the
                # smallest (4-tile) block instead of the 16-tile one
                tqb_order = (1, 2, 3, 0) if b == n_batches - 1 else range(bt_rt)
                for tqb in tqb_order:
                    with nc.named_scope(f"attn{b}_{tqb}"):
                        if pending_oproj:
                            queue_oproj(*pending_oproj.pop())
                        if not OPROJ_FILLER:
                            while filler:
                                pop_filler()
                        q0 = r0 + tqb * RT
                        n_tk = (tqb + 1) * (RT // 128)
                        qt_tile = qTs[q0 // RT]
                        yps = [
                            psY.tile([65, RT], F32, name=f"yacc{h}", tag="y")
                            for h in range(2)
                        ]

                        def s_tile(tk, tqb=tqb, r0=r0, qt_tile=qt_tile):
                            k0 = r0 + tk * 128
                            diag = tk * 128 >= tqb * RT
                            st = psS.tile([128, 2, RT], F32, name="st", tag="s")
                            kt_tile = kTs[k0 // RT]
                            kk = k0 % RT
                            if not diag:
                                qc0 = 0
                                for h in range(2):
                                    hs = slice(64 * h, 64 * h + 64)
                                    nc.tensor.matmul(
                                        st[:, h, :],
                                        kt_tile[hs, kk : kk + 128],
                                        qt_tile[hs, :],
                                        start=True,
                                        stop=True,
                                        skip_group_check=True,
                                    )
                            else:
                                # block-diagonal tile: only columns (queries)
                                # >= off*128 attend to any key here; the
                                # 128-wide triangle gets -50 via one L/E matmul
                                off = tk - tqb * (RT // 128)
                                qc0 = off * 128
                                for h in range(2):
                                    hs = slice(64 * h, 64 * h + 64)
                                    # causal triangle handled post-exp on DVE
                                    # (0/1 multiply), so one matmul suffices
                                    nc.tensor.matmul(
                                        st[:, h, qc0:],
                                        kt_tile[hs, kk : kk + 128],
                                        qt_tile[hs, qc0:],
                                        start=True,
                                        stop=True,
                                        skip_group_check=True,
                                    )
                            return st, qc0

                        cur = s_tile(0) if SBY else None
                        for tk in range(n_tk):
                            st, qc0 = cur if SBY else s_tile(tk)
                            diag = tk * 128 >= tqb * RT
                            pt = ptp.tile([128, 2, RT], BF16, name="pt")
                            nc.scalar.activation(
                                pt[:, :, qc0:],
                                st[:, :, qc0:],
                                mybir.ActivationFunctionType.Exp,
                            )
                            if diag:
                                for h in range(2):
                                    nc.vector.tensor_mul(
                                        pt[:, h, qc0 : qc0 + 128],
                                        pt[:, h, qc0 : qc0 + 128],
                                        tri01[:],
                                    )
                            # keep the PE fed while ACT runs exp: next tile's
                            # S matmuls, then one filler closure
                            if SBY and tk + 1 < n_tk:
                                cur = s_tile(tk + 1)
                            if OPROJ_FILLER or PROJ_FILLER:
                                pop_filler()
                            for h in range(2):
                                if not diag:
                                    nc.tensor.matmul(
                                        yps[h][:],
                                        v_aug[:, h, tk, :],
                                        pt[:, h, :],
                                        start=(tk == 0),
                                        stop=False,
                                        skip_group_check=True,
                                    )
                                else:
                                    # last writer for columns [qc0, qc0+128)
                                    nc.tensor.matmul(
                                        yps[h][:, qc0 : qc0 + 128],
                                        v_aug[:, h, tk, :],
                                        pt[:, h, qc0 : qc0 + 128],
                                        start=(tk == 0),
                                        stop=True,
                                        skip_group_check=True,
                                    )
                                    if qc0 + 128 < RT:
                                        nc.tensor.matmul(
                                            yps[h][:, qc0 + 128 :],
                                            v_aug[:, h, tk, :],
                                            pt[:, h, qc0 + 128 :],
                                            start=(tk == 0),
                                            stop=False,
                                            skip_group_check=True,
                                        )
                        # ---- normalize -> Y^T block [128, RT] ----
                        yt = ytp.tile([128, RT], BF16, name="yt")
                        for h in range(2):
                            ssum = nrm.tile([1, RT], F32, name="ssum")
                            nc.vector.tensor_copy(ssum[:], yps[h][64:65, :])
                            srow = nrm.tile([1, RT], F32, name="srow")
                            nc.vector.reciprocal_approx_fast(srow[:], ssum[:])
                            bc = nrm.tile([64, RT], F32, name="bc")
                            nc.gpsimd.partition_broadcast(bc[:], srow[:])
                            nc.vector.tensor_mul(
                                yt[64 * h : 64 * h + 64, :], yps[h][0:64, :], bc[:]
                            )
                    pending_oproj.append((yt, q0))
            while pending_oproj:
                queue_oproj(*pending_oproj.pop())
            while filler:
                pop_filler()
    nc.compile()
    return nc


def make_masks():
    """L/E such that (L.T @ E)[k, j] = NEG iff k > j (the within-tile
    128x128 causal triangle)."""
    L = np.zeros((128, 128), np.float32)
    for k in range(128):
        L[k, k + 1 :] = NEG
    L[127, :] = NEG
    E = np.zeros((128, 4, 128), np.float32)
    for oi in range(4):  # slot 0 is used for math; all 4 feed warmup matmuls
        for k in range(127):
            E[k, oi, k] = 1.0
    return L, E


def make_inputs_for_core(c, xT, Wq, bq, Wk, bk, Wv, bv, Wp, bp):
    import ml_dtypes

    bf = ml_dtypes.bfloat16
    cols = slice(c * 128, (c + 1) * 128)
    L, E = make_masks()
    idnp = np.zeros((128, 64), np.float32)
    for h in range(2):
        idnp[64 * h : 64 * h + 64] = np.eye(64, dtype=np.float32)

    return {
        "xT": xT,
        "wq": np.ascontiguousarray((np.asarray(Wq, np.float32)[:, cols] / 8.0)).astype(bf),
        "wk": np.ascontiguousarray(np.asarray(Wk, np.float32)[:, cols]).astype(bf),
        "wv": np.ascontiguousarray(np.asarray(Wv, np.float32)[:, cols]).astype(bf),
        "wp": np.ascontiguousarray(np.asarray(Wp, np.float32)[cols, :]).astype(bf),
        "bq": np.ascontiguousarray(np.asarray(bq, np.float32)[cols] / 8.0),
        "bk": np.ascontiguousarray(np.asarray(bk, np.float32)[cols]),
        "bv": np.ascontiguousarray(np.asarray(bv, np.float32)[cols]),
        "Lm": L.astype(bf),
        "Em": E.astype(bf),
        "ident": idnp.astype(bf),
        "tri01": np.tril(np.ones((128, 128), np.float32)).T.astype(bf),
        "onescol": np.ones((128, 2 * 2 * (T // 128)), bf),
    }


def kernel(x, Wq, bq, Wk, bk, Wv, bv, Wp, bp, _nc_cache={}, **run_kwargs):
    import ml_dtypes

    n_batches = B
    if "nc" not in _nc_cache:
        _nc_cache["nc"] = build_kernel(n_batches)
    nc = _nc_cache["nc"]
    xT = np.ascontiguousarray(
        np.asarray(x, np.float32).reshape(B * T, C).T
    ).astype(ml_dtypes.bfloat16)
    in_maps = [
        make_inputs_for_core(c, xT, Wq, bq, Wk, bk, Wv, bv, Wp, bp)
        for c in range(N_CORES)
    ]
    res = run_bass_kernel_spmd(nc, in_maps, core_ids=list(range(N_CORES)), **run_kwargs)
    out = np.zeros((B * T, C), np.float32)
    for r in res.results:
        out += np.asarray(r["out"], np.float32)
    out += np.asarray(bp, np.float32)[None, :]
    if run_kwargs.get("trace"):
        kernel.last_result = res
    return out.reshape(B, T, C)
